# revision 50
# baseline (speedup 1.0000x reference)
"""Trainium2 8-core kernel for nn_Attention_21345987461594.

Multi-head attention: B=2, S=4096, E=512, H=8 heads, D=64.
  qkv = x @ w_qkv + b_qkv ; per-head softmax(q k^T / sqrt(D)) v ; out proj.

Sharding: 16 (batch, head) pairs -> 2 heads per core (core c: batch c//4,
heads 2*(c%4), 2*(c%4)+1). No collectives: each core computes a partial
out-projection (rows of w_out for its heads) and the host sums the 4
partials per batch. All matmuls run in bf16 (f32 PSUM accumulate);
softmax skips max-subtraction (scores ~ N(0,1) after 1/sqrt(D) scaling,
exp is safely bounded) and the denominator is fused into the PV matmul
as an extra all-ones column of V.

Device layout is "transposed": scores are computed as S^T[key, q] so the
exp output feeds the PV matmul directly as the moving operand; the
out-projection then produces out^T[e, q], stored transposed in DRAM and
un-transposed on the host during the gather.
"""

import sys

if "/opt/trn_rl_repo" not in sys.path:
    sys.path.insert(0, "/opt/trn_rl_repo")

import numpy as np
import ml_dtypes

import concourse.bass as bass
import concourse.tile as tile
from concourse import bacc, mybir
from concourse.bass_utils import run_bass_kernel_spmd
from concourse.masks import make_identity

BF16 = mybir.dt.bfloat16
F32 = mybir.dt.float32

B, S, E, H = 2, 4096, 512, 8
D = E // H          # 64
HPC = 2             # heads per core
N_CORES = 8
QB = 512            # query block (free dim of score matmuls)
N_QB = S // QB      # 8
CH = 128            # key chunk
N_CH = S // CH      # 32
GRP = 3             # score chunks exp'd per ACT instruction

# fused [V | 1] stationary layout: per key-chunk, 65 cols per head
VW = HPC * (D + 1)  # 130


def _build():
    nc = bacc.Bacc("TRN2", target_bir_lowering=False)

    xt_ext = nc.declare_dram_parameter("xt", [E, S], BF16, isOutput=False)
    wqkv_ext = nc.declare_dram_parameter("wqkv", [E, 3 * HPC * D], BF16, isOutput=False)
    bqkv_ext = nc.declare_dram_parameter("bqkv", [3 * HPC * D, 1], F32, isOutput=False)
    wout_ext = nc.declare_dram_parameter("wout", [HPC * D, E], BF16, isOutput=False)
    out_ext = nc.declare_dram_parameter("out", [E, S], F32, isOutput=True)
    # DRAM bounce for the softmax-reciprocal partition broadcast
    dn_scr = [nc.dram_tensor(f"dnscr{i}", [HPC, QB], F32) for i in range(2)]

    FW = HPC * D  # 128, qkv projection tile width per ft

    with tile.TileContext(nc) as tc:
        with (
            tc.tile_pool(name="consts", bufs=1) as consts,
            tc.tile_pool(name="pt_pool", bufs=8) as pt_pool,
            tc.tile_pool(name="attn_pool", bufs=2) as attn_pool,
            tc.tile_pool(name="ot_pool", bufs=4) as ot_pool,
            tc.tile_pool(name="sm_pool", bufs=2) as sm_pool,
            tc.tile_pool(name="psum_sc", bufs=2, space="PSUM") as psum_sc,
            tc.tile_pool(name="psum_pv", bufs=2, space="PSUM") as psum_pv,
        ):
            # ---- persistent SBUF tensors ----
            xt_sb = [consts.tile([128, S], BF16, name=f"xt{e}") for e in range(4)]
            wq_sb = [consts.tile([128, 3 * FW], BF16, name=f"wq{e}") for e in range(4)]
            wout_sb = consts.tile([128, E], BF16, name="wout")
            b_t = [consts.tile([128, 1], F32, name=f"bq{f}") for f in range(3)]
            qT = consts.tile([128, S], BF16, name="qT")
            kT = consts.tile([128, S], BF16, name="kT")
            vT = consts.tile([128, S], BF16, name="vT")
            V_sb = consts.tile([128, N_CH * VW], BF16, name="V")
            ident_bf = consts.tile([128, 128], BF16, name="ident")

            # ---- loads / constants ----
            dma_engines = (nc.sync, nc.scalar)
            for e in range(4):
                dma_engines[e % 2].dma_start(
                    out=xt_sb[e][:, 0:QB], in_=xt_ext[e * 128 : (e + 1) * 128, 0:QB]
                )
            for e in range(4):
                dma_engines[e % 2].dma_start(
                    out=wq_sb[e], in_=wqkv_ext[e * 128 : (e + 1) * 128, :]
                )
            nc.sync.dma_start(out=wout_sb, in_=wout_ext[:, :])
            for f in range(3):
                nc.scalar.dma_start(
                    out=b_t[f], in_=bqkv_ext[f * FW : (f + 1) * FW, :]
                )
            make_identity(nc, ident_bf)
            V_view = V_sb.rearrange("p (c w) -> p c w", w=VW)
            nc.vector.memset(V_view[:, :, D : D + 1], 1.0)
            nc.vector.memset(V_view[:, :, VW - 1 : VW], 1.0)
            for tb in range(1, N_QB):
                for e in range(4):
                    dma_engines[(tb * 4 + e) % 2].dma_start(
                        out=xt_sb[e][:, tb * QB : (tb + 1) * QB],
                        in_=xt_ext[e * 128 : (e + 1) * 128, tb * QB : (tb + 1) * QB],
                    )

            # ---- qkv projection: (q|k|v)^T[f, t] ----
            dests = (qT, kT, vT)

            def proj(ft, tb):
                ps = psum_sc.tile([128, QB], F32, tag="sc", name=f"prj{ft}_{tb}")
                for e in range(4):
                    nc.tensor.matmul(
                        ps,
                        lhsT=wq_sb[e][:, ft * FW : (ft + 1) * FW],
                        rhs=xt_sb[e][:, tb * QB : (tb + 1) * QB],
                        start=(e == 0),
                        stop=(e == 3),
                    )
                nc.vector.tensor_scalar_add(
                    out=dests[ft][:, tb * QB : (tb + 1) * QB], in0=ps, scalar1=b_t[ft]
                )

            def vbuild(c):
                tp = psum_sc.tile([128, 128], BF16, tag="sc", name=f"tp{c}")
                nc.tensor.transpose(tp, vT[:, c * 128 : (c + 1) * 128], ident_bf)
                nc.vector.tensor_copy(out=V_view[:, c, 0:D], in_=tp[:, 0:D])
                nc.vector.tensor_copy(
                    out=V_view[:, c, D + 1 : VW - 1], in_=tp[:, D : 2 * D]
                )

            # upfront: k/q projections for block 0 only; the rest of the qkv
            # projection and the V-layout build drip into the attention
            # stream. Interleave k-projection (gates score groups), v
            # projection + V build (gates PV groups), then q (gates block 1+).
            proj(1, 0)
            proj(0, 0)
            extras = []
            for tb in range(N_QB):
                if tb >= 1:
                    extras.append((proj, 1, tb))
                extras.append((proj, 2, tb))
                for c in range(4 * tb, 4 * tb + 4):
                    extras.append((vbuild, c))
            for tb in range(1, N_QB):
                extras.append((proj, 0, tb))

            # ---- attention ----
            n_m = N_CH * HPC          # 64 score matmuls per query block
            n_grp = (n_m + GRP - 1) // GRP

            def emit_scores_exp(st, g):
                qb = st["qb"]
                size = min(GRP, n_m - g * GRP)
                sc = psum_sc.tile([128, GRP * QB], F32, tag="sc", name=f"sc{qb}_{g}")
                pt = pt_pool.tile([128, GRP * QB], BF16, tag="pt", name=f"pt{qb}_{g}")
                for s in range(size):
                    m = g * GRP + s
                    c, h = m >> 1, m & 1
                    nc.tensor.matmul(
                        sc[:, s * QB : (s + 1) * QB],
                        lhsT=kT[h * D : (h + 1) * D, c * CH : (c + 1) * CH],
                        rhs=qT[h * D : (h + 1) * D, qb * QB : (qb + 1) * QB],
                        start=True,
                        stop=True,
                    )
                nc.scalar.activation(
                    out=pt[:, : size * QB],
                    in_=sc[:, : size * QB],
                    func=mybir.ActivationFunctionType.Exp,
                    scale=float(D) ** -0.5,
                )
                st["pts"][g] = pt

            def emit_pv(st, g):
                qb = st["qb"]
                if st["pv"] is None:
                    st["pv"] = [
                        psum_pv.tile([128, QB], F32, tag="pv", name=f"pv{qb}_{h}")
                        for h in range(HPC)
                    ]
                size = min(GRP, n_m - g * GRP)
                pt = st["pts"].pop(g)
                for s in range(size):
                    m = g * GRP + s
                    c, h = m >> 1, m & 1
                    nc.tensor.matmul(
                        st["pv"][h][0 : D + 1, :],
                        lhsT=V_sb[:, c * VW + h * (D + 1) : c * VW + (h + 1) * (D + 1)],
                        rhs=pt[:, s * QB : (s + 1) * QB],
                        start=(c == 0),
                        stop=(c == N_CH - 1),
                    )

            def tail_step(st, step):
                qb = st["qb"]
                if step == 0:
                    # drain PSUM accumulators to SBUF
                    st["pvsb2"] = sm_pool.tile(
                        [128, QB], F32, tag="pvsb2", bufs=2, name=f"pvsb2_{qb}"
                    )
                    st["dn"] = [
                        sm_pool.tile([1, QB], F32, tag=f"dn{h}", bufs=2, name=f"dn{qb}_{h}")
                        for h in range(HPC)
                    ]
                    for h in range(HPC):
                        nc.vector.tensor_copy(
                            out=st["pvsb2"][h * D : (h + 1) * D, :],
                            in_=st["pv"][h][0:D, :],
                        )
                        nc.vector.tensor_copy(
                            out=st["dn"][h], in_=st["pv"][h][D : D + 1, :]
                        )
                elif step == 1:
                    # bounce denominators to DRAM (for partition broadcast)
                    for h in range(HPC):
                        nc.sync.dma_start(
                            out=dn_scr[qb % 2][h : h + 1, :], in_=st["dn"][h]
                        )
                elif step == 2:
                    # broadcast denominators across partitions via step-0
                    # DRAM->SBUF DMA
                    st["dnb"] = sm_pool.tile(
                        [128, QB], F32, tag="dnb", bufs=2, name=f"dnb{qb}"
                    )
                    for h in range(HPC):
                        row = dn_scr[qb % 2][h : h + 1, :]
                        src = bass.AP(
                            tensor=row.tensor,
                            offset=row.offset,
                            ap=[[0, D]] + list(row.ap),
                        )
                        nc.gpsimd.dma_start(
                            out=st["dnb"][h * D : (h + 1) * D, :], in_=src
                        )
                elif step == 3:
                    st["rcp"] = sm_pool.tile(
                        [128, QB], F32, tag="rcp", bufs=2, name=f"rcp{qb}"
                    )
                    nc.vector.reciprocal(out=st["rcp"], in_=st["dnb"])
                elif step == 4:
                    st["attnT"] = attn_pool.tile(
                        [128, QB], BF16, tag="attnT", name=f"attnT{qb}"
                    )
                    nc.vector.tensor_mul(
                        out=st["attnT"], in0=st["pvsb2"], in1=st["rcp"]
                    )
                else:
                    # step 5 / 6: out projection halves (partial, transposed)
                    pair = step - 5
                    op = psum_sc.tile(
                        [128, GRP * QB], F32, tag="sc", name=f"op{qb}_{pair}"
                    )
                    for k in range(2):
                        et = pair * 2 + k
                        nc.tensor.matmul(
                            op[:, k * QB : (k + 1) * QB],
                            lhsT=wout_sb[:, et * 128 : (et + 1) * 128],
                            rhs=st["attnT"],
                            start=True,
                            stop=True,
                        )
                    for k in range(2):
                        et = pair * 2 + k
                        ot = ot_pool.tile([128, QB], F32, tag="ot")
                        nc.vector.tensor_copy(out=ot, in_=op[:, k * QB : (k + 1) * QB])
                        nc.sync.dma_start(
                            out=out_ext[et * 128 : (et + 1) * 128, qb * QB : (qb + 1) * QB],
                            in_=ot,
                        )

            # Slot scheduler: each slot emits one score/exp group, drips
            # phase-1 extras, pays down the PV debt, and runs due tail steps
            # of finished blocks. Gating keeps emission order deadlock-free:
            # scores need their k/q projections emitted first, PV groups need
            # the V chunks they read built first.
            EXTRAS_PER_SLOT = 2
            TAIL_OFFS = (0, 1, 2, 4, 6, 8, 10)
            slot = 0
            pvq = []     # pending (st, g, emit_slot) PV groups, in order
            tails = []   # (st, step, due_slot)
            done = {"k": 0, "q": 0, "vb": 0}  # kproj tb<=, qproj tb<=, vbuilds

            def pop_extra():
                fn, *args = extras.pop(0)
                fn(*args)
                if fn is proj:
                    if args[0] == 1:
                        done["k"] = max(done["k"], args[1])
                    elif args[0] == 0:
                        done["q"] = max(done["q"], args[1])
                else:
                    done["vb"] += 1

            def pump_pv():
                for _ in range(2):
                    if not pvq:
                        return
                    s2, g2, es = pvq[0]
                    if slot < es + 2 or done["vb"] < min(N_CH, (3 * g2 + 2) // 2 + 1):
                        return
                    pvq.pop(0)
                    emit_pv(s2, g2)
                    if g2 == n_grp - 1:
                        for k, off in enumerate(TAIL_OFFS):
                            tails.append((s2, k, slot + off))
                        return  # drains must run before the next block's PV

            def pump_tails():
                while tails and tails[0][2] <= slot:
                    s2, k, _ = tails.pop(0)
                    tail_step(s2, k)

            for qb in range(N_QB):
                st = {"qb": qb, "pts": {}, "pv": None}
                for g in range(n_grp):
                    k_need = min(N_QB - 1, (3 * g + 2) // 8)
                    while extras and (done["k"] < k_need or done["q"] < qb):
                        pop_extra()
                    emit_scores_exp(st, g)
                    pvq.append((st, g, slot))
                    for _ in range(EXTRAS_PER_SLOT):
                        if extras:
                            pop_extra()
                    pump_pv()
                    pump_tails()
                    slot += 1
            while extras:
                pop_extra()
            while pvq or tails:
                pump_pv()
                pump_tails()
                slot += 1

    nc.compile()
    return nc


_NC = None
LAST = {}


def _get_nc():
    global _NC
    if _NC is None:
        _NC = _build()
    return _NC


def kernel(x, w_qkv, b_qkv, w_out, b_out):
    x = np.asarray(x, dtype=np.float32)
    w_qkv = np.asarray(w_qkv, dtype=np.float32)
    b_qkv = np.asarray(b_qkv, dtype=np.float32)
    w_out = np.asarray(w_out, dtype=np.float32)
    b_out = np.asarray(b_out, dtype=np.float32)

    bf = ml_dtypes.bfloat16
    in_maps = []
    for c in range(N_CORES):
        b = c // 4
        h0 = (c % 4) * HPC * D  # first head's column offset (2 heads = 128 cols)
        w_slice = np.concatenate(
            [w_qkv[:, j * E + h0 : j * E + h0 + HPC * D] for j in range(3)], axis=1
        )
        b_slice = np.concatenate(
            [b_qkv[j * E + h0 : j * E + h0 + HPC * D] for j in range(3)]
        )[:, None]
        in_maps.append(
            {
                "xt": np.ascontiguousarray(x[b].T).astype(bf),
                "wqkv": np.ascontiguousarray(w_slice).astype(bf),
                "bqkv": np.ascontiguousarray(b_slice.astype(np.float32)),
                "wout": np.ascontiguousarray(w_out[h0 : h0 + HPC * D, :]).astype(bf),
            }
        )

    res = run_bass_kernel_spmd(_get_nc(), in_maps, list(range(N_CORES)))
    LAST["exec_time_ns"] = res.exec_time_ns
    LAST["res"] = res

    out = np.empty((B, S, E), dtype=np.float32)
    for b in range(B):
        acc = res.results[4 * b]["out"].astype(np.float32)
        for c in range(4 * b + 1, 4 * b + 4):
            acc = acc + res.results[c]["out"]
        out[b] = acc.T + b_out[None, :]
    return out


# revision 54
# speedup vs baseline: 1.0144x; 1.0144x over previous
"""Trainium2 8-core kernel for nn_Attention_21345987461594.

Multi-head attention: B=2, S=4096, E=512, H=8 heads, D=64.
  qkv = x @ w_qkv + b_qkv ; per-head softmax(q k^T / sqrt(D)) v ; out proj.

Sharding: 16 (batch, head) pairs -> 2 heads per core (core c: batch c//4,
heads 2*(c%4), 2*(c%4)+1). No collectives: each core computes a partial
out-projection (rows of w_out for its heads) and the host sums the 4
partials per batch. All matmuls run in bf16 (f32 PSUM accumulate);
softmax skips max-subtraction (scores ~ N(0,1) after 1/sqrt(D) scaling,
exp is safely bounded) and the denominator is fused into the PV matmul
as an extra all-ones column of V.

Device layout is "transposed": scores are computed as S^T[key, q] so the
exp output feeds the PV matmul directly as the moving operand; the
out-projection then produces out^T[e, q], stored transposed in DRAM and
un-transposed on the host during the gather.
"""

import sys

if "/opt/trn_rl_repo" not in sys.path:
    sys.path.insert(0, "/opt/trn_rl_repo")

import numpy as np
import ml_dtypes

import concourse.bass as bass
import concourse.tile as tile
from concourse import bacc, mybir
from concourse.bass_utils import run_bass_kernel_spmd
from concourse.masks import make_identity

BF16 = mybir.dt.bfloat16
F32 = mybir.dt.float32

B, S, E, H = 2, 4096, 512, 8
D = E // H          # 64
HPC = 2             # heads per core
N_CORES = 8
QB = 512            # query block (free dim of score matmuls)
N_QB = S // QB      # 8
CH = 128            # key chunk
N_CH = S // CH      # 32
GRP = 3             # score chunks exp'd per ACT instruction

# fused [V | 1] stationary layout: per key-chunk, 65 cols per head
VW = HPC * (D + 1)  # 130


def _build():
    nc = bacc.Bacc("TRN2", target_bir_lowering=False)

    xt_ext = nc.declare_dram_parameter("xt", [E, S], BF16, isOutput=False)
    wqkv_ext = nc.declare_dram_parameter("wqkv", [E, 3 * HPC * D], BF16, isOutput=False)
    bqkv_ext = nc.declare_dram_parameter("bqkv", [3 * HPC * D, 1], F32, isOutput=False)
    wout_ext = nc.declare_dram_parameter("wout", [HPC * D, E], BF16, isOutput=False)
    out_ext = nc.declare_dram_parameter("out", [E, S], F32, isOutput=True)
    # DRAM bounce for the softmax-reciprocal partition broadcast
    dn_scr = [nc.dram_tensor(f"dnscr{i}", [HPC, QB], F32) for i in range(2)]

    FW = HPC * D  # 128, qkv projection tile width per ft

    with tile.TileContext(nc) as tc:
        with (
            tc.tile_pool(name="consts", bufs=1) as consts,
            tc.tile_pool(name="pt_pool", bufs=8) as pt_pool,
            tc.tile_pool(name="attn_pool", bufs=2) as attn_pool,
            tc.tile_pool(name="ot_pool", bufs=4) as ot_pool,
            tc.tile_pool(name="sm_pool", bufs=2) as sm_pool,
            tc.tile_pool(name="psum_sc", bufs=2, space="PSUM") as psum_sc,
            tc.tile_pool(name="psum_pv", bufs=2, space="PSUM") as psum_pv,
        ):
            # ---- persistent SBUF tensors ----
            xt_sb = [consts.tile([128, S], BF16, name=f"xt{e}") for e in range(4)]
            wq_sb = [consts.tile([128, 3 * FW], BF16, name=f"wq{e}") for e in range(4)]
            wout_sb = consts.tile([128, E], BF16, name="wout")
            b_t = [consts.tile([128, 1], F32, name=f"bq{f}") for f in range(3)]
            qT = consts.tile([128, S], BF16, name="qT")
            kT = consts.tile([128, S], BF16, name="kT")
            vT = consts.tile([128, S], BF16, name="vT")
            V_sb = consts.tile([128, N_CH * VW], BF16, name="V")
            ident_bf = consts.tile([128, 128], BF16, name="ident")

            # ---- loads / constants ----
            dma_engines = (nc.sync, nc.scalar)
            for e in range(4):
                dma_engines[e % 2].dma_start(
                    out=xt_sb[e][:, 0:QB], in_=xt_ext[e * 128 : (e + 1) * 128, 0:QB]
                )
            for e in range(4):
                dma_engines[e % 2].dma_start(
                    out=wq_sb[e], in_=wqkv_ext[e * 128 : (e + 1) * 128, :]
                )
            nc.sync.dma_start(out=wout_sb, in_=wout_ext[:, :])
            for f in range(3):
                nc.scalar.dma_start(
                    out=b_t[f], in_=bqkv_ext[f * FW : (f + 1) * FW, :]
                )
            make_identity(nc, ident_bf)
            V_view = V_sb.rearrange("p (c w) -> p c w", w=VW)
            nc.vector.memset(V_view[:, :, D : D + 1], 1.0)
            nc.vector.memset(V_view[:, :, VW - 1 : VW], 1.0)
            for tb in range(1, N_QB):
                for e in range(4):
                    dma_engines[(tb * 4 + e) % 2].dma_start(
                        out=xt_sb[e][:, tb * QB : (tb + 1) * QB],
                        in_=xt_ext[e * 128 : (e + 1) * 128, tb * QB : (tb + 1) * QB],
                    )

            # ---- qkv projection: (q|k|v)^T[f, t] ----
            dests = (qT, kT, vT)

            def proj(ft, tbs):
                # batch up to 3 token-blocks per stationary weight load
                ps = psum_sc.tile(
                    [128, GRP * QB], F32, tag="sc", name=f"prj{ft}_{tbs[0]}"
                )
                for e in range(4):
                    for i, tb in enumerate(tbs):
                        nc.tensor.matmul(
                            ps[:, i * QB : (i + 1) * QB],
                            lhsT=wq_sb[e][:, ft * FW : (ft + 1) * FW],
                            rhs=xt_sb[e][:, tb * QB : (tb + 1) * QB],
                            start=(e == 0),
                            stop=(e == 3),
                        )
                for i, tb in enumerate(tbs):
                    nc.vector.tensor_scalar_add(
                        out=dests[ft][:, tb * QB : (tb + 1) * QB],
                        in0=ps[:, i * QB : (i + 1) * QB],
                        scalar1=b_t[ft],
                    )

            def vbuild(c0):
                # four key-chunk transposes per PSUM slot
                tp = psum_sc.tile([128, GRP * QB], BF16, tag="sc", name=f"tp{c0}")
                for i in range(4):
                    c = c0 + i
                    nc.tensor.transpose(
                        tp[:, i * 128 : (i + 1) * 128],
                        vT[:, c * 128 : (c + 1) * 128],
                        ident_bf,
                    )
                for i in range(4):
                    c = c0 + i
                    nc.vector.tensor_copy(
                        out=V_view[:, c, 0:D], in_=tp[:, i * 128 : i * 128 + D]
                    )
                    nc.vector.tensor_copy(
                        out=V_view[:, c, D + 1 : VW - 1],
                        in_=tp[:, i * 128 + D : i * 128 + 2 * D],
                    )

            # upfront: k/q projections for block 0 only; the rest of the qkv
            # projection and the V-layout build drip into the attention
            # stream. Interleave k-projection (gates score groups), v
            # projection + V build (gates PV groups), then q (gates block 1+).
            proj(1, [0])
            proj(0, [0])
            extras = [
                (proj, 1, [1, 2, 3]),
                (proj, 2, [0, 1, 2]),
                (vbuild, 0),
                (vbuild, 4),
                (proj, 1, [4, 5, 6]),
                (proj, 2, [3, 4, 5]),
                (vbuild, 8),
                (vbuild, 12),
                (proj, 1, [7]),
                (proj, 2, [6, 7]),
                (vbuild, 16),
                (vbuild, 20),
                (vbuild, 24),
                (vbuild, 28),
                (proj, 0, [1, 2, 3]),
                (proj, 0, [4, 5, 6]),
                (proj, 0, [7]),
            ]

            # ---- attention ----
            n_m = N_CH * HPC          # 64 score matmuls per query block
            n_grp = (n_m + GRP - 1) // GRP

            def emit_scores_exp(st, g):
                qb = st["qb"]
                size = min(GRP, n_m - g * GRP)
                sc = psum_sc.tile([128, GRP * QB], F32, tag="sc", name=f"sc{qb}_{g}")
                pt = pt_pool.tile([128, GRP * QB], BF16, tag="pt", name=f"pt{qb}_{g}")
                for s in range(size):
                    m = g * GRP + s
                    c, h = m >> 1, m & 1
                    nc.tensor.matmul(
                        sc[:, s * QB : (s + 1) * QB],
                        lhsT=kT[h * D : (h + 1) * D, c * CH : (c + 1) * CH],
                        rhs=qT[h * D : (h + 1) * D, qb * QB : (qb + 1) * QB],
                        start=True,
                        stop=True,
                    )
                nc.scalar.activation(
                    out=pt[:, : size * QB],
                    in_=sc[:, : size * QB],
                    func=mybir.ActivationFunctionType.Exp,
                    scale=float(D) ** -0.5,
                )
                st["pts"][g] = pt

            def emit_pv(st, g):
                qb = st["qb"]
                if st["pv"] is None:
                    st["pv"] = [
                        psum_pv.tile([128, QB], F32, tag="pv", name=f"pv{qb}_{h}")
                        for h in range(HPC)
                    ]
                size = min(GRP, n_m - g * GRP)
                pt = st["pts"].pop(g)
                for s in range(size):
                    m = g * GRP + s
                    c, h = m >> 1, m & 1
                    nc.tensor.matmul(
                        st["pv"][h][0 : D + 1, :],
                        lhsT=V_sb[:, c * VW + h * (D + 1) : c * VW + (h + 1) * (D + 1)],
                        rhs=pt[:, s * QB : (s + 1) * QB],
                        start=(c == 0),
                        stop=(c == N_CH - 1),
                    )

            def tail_step(st, step):
                qb = st["qb"]
                if step == 0:
                    # drain PSUM accumulators to SBUF
                    st["pvsb2"] = sm_pool.tile(
                        [128, QB], F32, tag="pvsb2", bufs=2, name=f"pvsb2_{qb}"
                    )
                    st["dn"] = [
                        sm_pool.tile([1, QB], F32, tag=f"dn{h}", bufs=2, name=f"dn{qb}_{h}")
                        for h in range(HPC)
                    ]
                    for h in range(HPC):
                        nc.vector.tensor_copy(
                            out=st["pvsb2"][h * D : (h + 1) * D, :],
                            in_=st["pv"][h][0:D, :],
                        )
                        nc.vector.tensor_copy(
                            out=st["dn"][h], in_=st["pv"][h][D : D + 1, :]
                        )
                elif step == 1:
                    # bounce denominators to DRAM (for partition broadcast)
                    for h in range(HPC):
                        nc.sync.dma_start(
                            out=dn_scr[qb % 2][h : h + 1, :], in_=st["dn"][h]
                        )
                elif step == 2:
                    # broadcast denominators across partitions via step-0
                    # DRAM->SBUF DMA
                    st["dnb"] = sm_pool.tile(
                        [128, QB], F32, tag="dnb", bufs=2, name=f"dnb{qb}"
                    )
                    for h in range(HPC):
                        row = dn_scr[qb % 2][h : h + 1, :]
                        src = bass.AP(
                            tensor=row.tensor,
                            offset=row.offset,
                            ap=[[0, D]] + list(row.ap),
                        )
                        nc.gpsimd.dma_start(
                            out=st["dnb"][h * D : (h + 1) * D, :], in_=src
                        )
                elif step == 3:
                    st["rcp"] = sm_pool.tile(
                        [128, QB], F32, tag="rcp", bufs=2, name=f"rcp{qb}"
                    )
                    nc.vector.reciprocal(out=st["rcp"], in_=st["dnb"])
                elif step == 4:
                    st["attnT"] = attn_pool.tile(
                        [128, QB], BF16, tag="attnT", name=f"attnT{qb}"
                    )
                    nc.vector.tensor_mul(
                        out=st["attnT"], in0=st["pvsb2"], in1=st["rcp"]
                    )
                else:
                    # step 5 / 6: out projection halves (partial, transposed)
                    pair = step - 5
                    op = psum_sc.tile(
                        [128, GRP * QB], F32, tag="sc", name=f"op{qb}_{pair}"
                    )
                    for k in range(2):
                        et = pair * 2 + k
                        nc.tensor.matmul(
                            op[:, k * QB : (k + 1) * QB],
                            lhsT=wout_sb[:, et * 128 : (et + 1) * 128],
                            rhs=st["attnT"],
                            start=True,
                            stop=True,
                        )
                    for k in range(2):
                        et = pair * 2 + k
                        ot = ot_pool.tile([128, QB], F32, tag="ot")
                        nc.vector.tensor_copy(out=ot, in_=op[:, k * QB : (k + 1) * QB])
                        nc.sync.dma_start(
                            out=out_ext[et * 128 : (et + 1) * 128, qb * QB : (qb + 1) * QB],
                            in_=ot,
                        )

            # Slot scheduler: each slot emits one score/exp group, drips
            # phase-1 extras, pays down the PV debt, and runs due tail steps
            # of finished blocks. Gating keeps emission order deadlock-free:
            # scores need their k/q projections emitted first, PV groups need
            # the V chunks they read built first.
            EXTRAS_PER_SLOT = 1
            TAIL_OFFS = (0, 1, 2, 4, 6, 8, 10)
            slot = 0
            pvq = []     # pending (st, g, emit_slot) PV groups, in order
            tails = []   # (st, step, due_slot)
            done = {"k": 0, "q": 0, "vb": 0}  # kproj tb<=, qproj tb<=, vbuilds

            def pop_extra():
                fn, *args = extras.pop(0)
                fn(*args)
                if fn is proj:
                    if args[0] == 1:
                        done["k"] = max(done["k"], max(args[1]))
                    elif args[0] == 0:
                        done["q"] = max(done["q"], max(args[1]))
                else:
                    done["vb"] += 4

            def pump_pv():
                for _ in range(2):
                    if not pvq:
                        return
                    s2, g2, es = pvq[0]
                    if slot < es + 2 or done["vb"] < min(N_CH, (3 * g2 + 2) // 2 + 1):
                        return
                    pvq.pop(0)
                    emit_pv(s2, g2)
                    if g2 == n_grp - 1:
                        for k, off in enumerate(TAIL_OFFS):
                            tails.append((s2, k, slot + off))
                        return  # drains must run before the next block's PV

            def pump_tails():
                while tails and tails[0][2] <= slot:
                    s2, k, _ = tails.pop(0)
                    tail_step(s2, k)

            for qb in range(N_QB):
                st = {"qb": qb, "pts": {}, "pv": None}
                for g in range(n_grp):
                    k_need = min(N_QB - 1, (3 * g + 2) // 8)
                    while extras and (done["k"] < k_need or done["q"] < qb):
                        pop_extra()
                    emit_scores_exp(st, g)
                    pvq.append((st, g, slot))
                    for _ in range(EXTRAS_PER_SLOT):
                        if extras:
                            pop_extra()
                    pump_pv()
                    pump_tails()
                    slot += 1
            while extras:
                pop_extra()
            while pvq or tails:
                pump_pv()
                pump_tails()
                slot += 1

    nc.compile()
    return nc


_NC = None
LAST = {}


def _get_nc():
    global _NC
    if _NC is None:
        _NC = _build()
    return _NC


def kernel(x, w_qkv, b_qkv, w_out, b_out):
    x = np.asarray(x, dtype=np.float32)
    w_qkv = np.asarray(w_qkv, dtype=np.float32)
    b_qkv = np.asarray(b_qkv, dtype=np.float32)
    w_out = np.asarray(w_out, dtype=np.float32)
    b_out = np.asarray(b_out, dtype=np.float32)

    bf = ml_dtypes.bfloat16
    in_maps = []
    for c in range(N_CORES):
        b = c // 4
        h0 = (c % 4) * HPC * D  # first head's column offset (2 heads = 128 cols)
        w_slice = np.concatenate(
            [w_qkv[:, j * E + h0 : j * E + h0 + HPC * D] for j in range(3)], axis=1
        )
        b_slice = np.concatenate(
            [b_qkv[j * E + h0 : j * E + h0 + HPC * D] for j in range(3)]
        )[:, None]
        in_maps.append(
            {
                "xt": np.ascontiguousarray(x[b].T).astype(bf),
                "wqkv": np.ascontiguousarray(w_slice).astype(bf),
                "bqkv": np.ascontiguousarray(b_slice.astype(np.float32)),
                "wout": np.ascontiguousarray(w_out[h0 : h0 + HPC * D, :]).astype(bf),
            }
        )

    res = run_bass_kernel_spmd(_get_nc(), in_maps, list(range(N_CORES)))
    LAST["exec_time_ns"] = res.exec_time_ns
    LAST["res"] = res

    out = np.empty((B, S, E), dtype=np.float32)
    for b in range(B):
        acc = res.results[4 * b]["out"].astype(np.float32)
        for c in range(4 * b + 1, 4 * b + 4):
            acc = acc + res.results[c]["out"]
        out[b] = acc.T + b_out[None, :]
    return out


# revision 55
# speedup vs baseline: 1.0145x; 1.0001x over previous
"""Trainium2 8-core kernel for nn_Attention_21345987461594.

Multi-head attention: B=2, S=4096, E=512, H=8 heads, D=64.
  qkv = x @ w_qkv + b_qkv ; per-head softmax(q k^T / sqrt(D)) v ; out proj.

Sharding: 16 (batch, head) pairs -> 2 heads per core (core c: batch c//4,
heads 2*(c%4), 2*(c%4)+1). No collectives: each core computes a partial
out-projection (rows of w_out for its heads) and the host sums the 4
partials per batch. All matmuls run in bf16 (f32 PSUM accumulate);
softmax skips max-subtraction (scores ~ N(0,1) after 1/sqrt(D) scaling,
exp is safely bounded) and the denominator is fused into the PV matmul
as an extra all-ones column of V.

Device layout is "transposed": scores are computed as S^T[key, q] so the
exp output feeds the PV matmul directly as the moving operand; the
out-projection then produces out^T[e, q], stored transposed in DRAM and
un-transposed on the host during the gather.
"""

import sys

if "/opt/trn_rl_repo" not in sys.path:
    sys.path.insert(0, "/opt/trn_rl_repo")

import numpy as np
import ml_dtypes

import concourse.bass as bass
import concourse.tile as tile
from concourse import bacc, mybir
from concourse.bass_utils import run_bass_kernel_spmd
from concourse.masks import make_identity

BF16 = mybir.dt.bfloat16
F32 = mybir.dt.float32

B, S, E, H = 2, 4096, 512, 8
D = E // H          # 64
HPC = 2             # heads per core
N_CORES = 8
QB = 512            # query block (free dim of score matmuls)
N_QB = S // QB      # 8
CH = 128            # key chunk
N_CH = S // CH      # 32
GRP = 3             # score chunks exp'd per ACT instruction

# fused [V | 1] stationary layout: per key-chunk, 65 cols per head
VW = HPC * (D + 1)  # 130


def _build():
    nc = bacc.Bacc("TRN2", target_bir_lowering=False)

    xt_ext = nc.declare_dram_parameter("xt", [E, S], BF16, isOutput=False)
    wqkv_ext = nc.declare_dram_parameter("wqkv", [E, 3 * HPC * D], BF16, isOutput=False)
    bqkv_ext = nc.declare_dram_parameter("bqkv", [3 * HPC * D, 1], F32, isOutput=False)
    wout_ext = nc.declare_dram_parameter("wout", [HPC * D, E], BF16, isOutput=False)
    out_ext = nc.declare_dram_parameter("out", [E, S], F32, isOutput=True)
    # DRAM bounce for the softmax-reciprocal partition broadcast
    dn_scr = [nc.dram_tensor(f"dnscr{i}", [HPC, QB], F32) for i in range(2)]

    FW = HPC * D  # 128, qkv projection tile width per ft

    with tile.TileContext(nc) as tc:
        with (
            tc.tile_pool(name="consts", bufs=1) as consts,
            tc.tile_pool(name="pt_pool", bufs=8) as pt_pool,
            tc.tile_pool(name="attn_pool", bufs=2) as attn_pool,
            tc.tile_pool(name="ot_pool", bufs=4) as ot_pool,
            tc.tile_pool(name="sm_pool", bufs=2) as sm_pool,
            tc.tile_pool(name="psum_sc", bufs=2, space="PSUM") as psum_sc,
            tc.tile_pool(name="psum_pv", bufs=2, space="PSUM") as psum_pv,
        ):
            # ---- persistent SBUF tensors ----
            xt_sb = [consts.tile([128, S], BF16, name=f"xt{e}") for e in range(4)]
            wq_sb = [consts.tile([128, 3 * FW], BF16, name=f"wq{e}") for e in range(4)]
            wout_sb = consts.tile([128, E], BF16, name="wout")
            b_t = [consts.tile([128, 1], F32, name=f"bq{f}") for f in range(3)]
            qT = consts.tile([128, S], BF16, name="qT")
            kT = consts.tile([128, S], BF16, name="kT")
            vT = consts.tile([128, S], BF16, name="vT")
            V_sb = consts.tile([128, N_CH * VW], BF16, name="V")
            ident_bf = consts.tile([128, 128], BF16, name="ident")

            # ---- loads / constants ----
            dma_engines = (nc.sync, nc.scalar)
            for e in range(4):
                dma_engines[e % 2].dma_start(
                    out=xt_sb[e][:, 0:QB], in_=xt_ext[e * 128 : (e + 1) * 128, 0:QB]
                )
            for e in range(4):
                dma_engines[e % 2].dma_start(
                    out=wq_sb[e], in_=wqkv_ext[e * 128 : (e + 1) * 128, :]
                )
            nc.sync.dma_start(out=wout_sb, in_=wout_ext[:, :])
            for f in range(3):
                nc.scalar.dma_start(
                    out=b_t[f], in_=bqkv_ext[f * FW : (f + 1) * FW, :]
                )
            make_identity(nc, ident_bf)
            V_view = V_sb.rearrange("p (c w) -> p c w", w=VW)
            junk = consts.tile([128, 256], BF16, name="junk")
            nc.vector.memset(junk, 0.5)
            nc.vector.memset(V_view[:, :, D : D + 1], 1.0)
            nc.vector.memset(V_view[:, :, VW - 1 : VW], 1.0)
            # warm the PE clock gate while the first DMAs land
            for w in range(24):
                wp = psum_pv.tile([128, 256], F32, tag="pv", name=f"warm{w}")
                nc.tensor.matmul(wp, lhsT=junk[:, 0:128], rhs=junk, start=True, stop=True)
            for tb in range(1, N_QB):
                for e in range(4):
                    dma_engines[(tb * 4 + e) % 2].dma_start(
                        out=xt_sb[e][:, tb * QB : (tb + 1) * QB],
                        in_=xt_ext[e * 128 : (e + 1) * 128, tb * QB : (tb + 1) * QB],
                    )

            # ---- qkv projection: (q|k|v)^T[f, t] ----
            dests = (qT, kT, vT)

            def proj(ft, tbs):
                # batch up to 3 token-blocks per stationary weight load
                ps = psum_sc.tile(
                    [128, GRP * QB], F32, tag="sc", name=f"prj{ft}_{tbs[0]}"
                )
                for e in range(4):
                    for i, tb in enumerate(tbs):
                        nc.tensor.matmul(
                            ps[:, i * QB : (i + 1) * QB],
                            lhsT=wq_sb[e][:, ft * FW : (ft + 1) * FW],
                            rhs=xt_sb[e][:, tb * QB : (tb + 1) * QB],
                            start=(e == 0),
                            stop=(e == 3),
                        )
                for i, tb in enumerate(tbs):
                    nc.vector.tensor_scalar_add(
                        out=dests[ft][:, tb * QB : (tb + 1) * QB],
                        in0=ps[:, i * QB : (i + 1) * QB],
                        scalar1=b_t[ft],
                    )

            def vbuild(c0):
                # four key-chunk transposes per PSUM slot
                tp = psum_sc.tile([128, GRP * QB], BF16, tag="sc", name=f"tp{c0}")
                for i in range(4):
                    c = c0 + i
                    nc.tensor.transpose(
                        tp[:, i * 128 : (i + 1) * 128],
                        vT[:, c * 128 : (c + 1) * 128],
                        ident_bf,
                    )
                for i in range(4):
                    c = c0 + i
                    nc.vector.tensor_copy(
                        out=V_view[:, c, 0:D], in_=tp[:, i * 128 : i * 128 + D]
                    )
                    nc.vector.tensor_copy(
                        out=V_view[:, c, D + 1 : VW - 1],
                        in_=tp[:, i * 128 + D : i * 128 + 2 * D],
                    )

            # upfront: k/q projections for block 0 only; the rest of the qkv
            # projection and the V-layout build drip into the attention
            # stream. Interleave k-projection (gates score groups), v
            # projection + V build (gates PV groups), then q (gates block 1+).
            proj(1, [0])
            proj(0, [0])
            extras = [
                (proj, 1, [1, 2, 3]),
                (proj, 2, [0, 1, 2]),
                (vbuild, 0),
                (vbuild, 4),
                (proj, 1, [4, 5, 6]),
                (proj, 2, [3, 4, 5]),
                (vbuild, 8),
                (vbuild, 12),
                (proj, 1, [7]),
                (proj, 2, [6, 7]),
                (vbuild, 16),
                (vbuild, 20),
                (vbuild, 24),
                (vbuild, 28),
                (proj, 0, [1, 2, 3]),
                (proj, 0, [4, 5, 6]),
                (proj, 0, [7]),
            ]

            # ---- attention ----
            n_m = N_CH * HPC          # 64 score matmuls per query block
            n_grp = (n_m + GRP - 1) // GRP

            def emit_scores_exp(st, g):
                qb = st["qb"]
                size = min(GRP, n_m - g * GRP)
                sc = psum_sc.tile([128, GRP * QB], F32, tag="sc", name=f"sc{qb}_{g}")
                pt = pt_pool.tile([128, GRP * QB], BF16, tag="pt", name=f"pt{qb}_{g}")
                for s in range(size):
                    m = g * GRP + s
                    c, h = m >> 1, m & 1
                    nc.tensor.matmul(
                        sc[:, s * QB : (s + 1) * QB],
                        lhsT=kT[h * D : (h + 1) * D, c * CH : (c + 1) * CH],
                        rhs=qT[h * D : (h + 1) * D, qb * QB : (qb + 1) * QB],
                        start=True,
                        stop=True,
                    )
                nc.scalar.activation(
                    out=pt[:, : size * QB],
                    in_=sc[:, : size * QB],
                    func=mybir.ActivationFunctionType.Exp,
                    scale=float(D) ** -0.5,
                )
                st["pts"][g] = pt

            def emit_pv(st, g):
                qb = st["qb"]
                if st["pv"] is None:
                    st["pv"] = [
                        psum_pv.tile([128, QB], F32, tag="pv", name=f"pv{qb}_{h}")
                        for h in range(HPC)
                    ]
                size = min(GRP, n_m - g * GRP)
                pt = st["pts"].pop(g)
                for s in range(size):
                    m = g * GRP + s
                    c, h = m >> 1, m & 1
                    nc.tensor.matmul(
                        st["pv"][h][0 : D + 1, :],
                        lhsT=V_sb[:, c * VW + h * (D + 1) : c * VW + (h + 1) * (D + 1)],
                        rhs=pt[:, s * QB : (s + 1) * QB],
                        start=(c == 0),
                        stop=(c == N_CH - 1),
                    )

            def tail_step(st, step):
                qb = st["qb"]
                if step == 0:
                    # drain PSUM accumulators to SBUF
                    st["pvsb2"] = sm_pool.tile(
                        [128, QB], F32, tag="pvsb2", bufs=2, name=f"pvsb2_{qb}"
                    )
                    st["dn"] = [
                        sm_pool.tile([1, QB], F32, tag=f"dn{h}", bufs=2, name=f"dn{qb}_{h}")
                        for h in range(HPC)
                    ]
                    for h in range(HPC):
                        nc.vector.tensor_copy(
                            out=st["pvsb2"][h * D : (h + 1) * D, :],
                            in_=st["pv"][h][0:D, :],
                        )
                        nc.vector.tensor_copy(
                            out=st["dn"][h], in_=st["pv"][h][D : D + 1, :]
                        )
                elif step == 1:
                    # bounce denominators to DRAM (for partition broadcast)
                    for h in range(HPC):
                        nc.sync.dma_start(
                            out=dn_scr[qb % 2][h : h + 1, :], in_=st["dn"][h]
                        )
                elif step == 2:
                    # broadcast denominators across partitions via step-0
                    # DRAM->SBUF DMA
                    st["dnb"] = sm_pool.tile(
                        [128, QB], F32, tag="dnb", bufs=2, name=f"dnb{qb}"
                    )
                    for h in range(HPC):
                        row = dn_scr[qb % 2][h : h + 1, :]
                        src = bass.AP(
                            tensor=row.tensor,
                            offset=row.offset,
                            ap=[[0, D]] + list(row.ap),
                        )
                        nc.gpsimd.dma_start(
                            out=st["dnb"][h * D : (h + 1) * D, :], in_=src
                        )
                elif step == 3:
                    st["rcp"] = sm_pool.tile(
                        [128, QB], F32, tag="rcp", bufs=2, name=f"rcp{qb}"
                    )
                    nc.vector.reciprocal(out=st["rcp"], in_=st["dnb"])
                elif step == 4:
                    st["attnT"] = attn_pool.tile(
                        [128, QB], BF16, tag="attnT", name=f"attnT{qb}"
                    )
                    nc.vector.tensor_mul(
                        out=st["attnT"], in0=st["pvsb2"], in1=st["rcp"]
                    )
                else:
                    # step 5 / 6: out projection halves (partial, transposed)
                    pair = step - 5
                    op = psum_sc.tile(
                        [128, GRP * QB], F32, tag="sc", name=f"op{qb}_{pair}"
                    )
                    for k in range(2):
                        et = pair * 2 + k
                        nc.tensor.matmul(
                            op[:, k * QB : (k + 1) * QB],
                            lhsT=wout_sb[:, et * 128 : (et + 1) * 128],
                            rhs=st["attnT"],
                            start=True,
                            stop=True,
                        )
                    for k in range(2):
                        et = pair * 2 + k
                        ot = ot_pool.tile([128, QB], F32, tag="ot")
                        nc.vector.tensor_copy(out=ot, in_=op[:, k * QB : (k + 1) * QB])
                        nc.sync.dma_start(
                            out=out_ext[et * 128 : (et + 1) * 128, qb * QB : (qb + 1) * QB],
                            in_=ot,
                        )

            # Slot scheduler: each slot emits one score/exp group, drips
            # phase-1 extras, pays down the PV debt, and runs due tail steps
            # of finished blocks. Gating keeps emission order deadlock-free:
            # scores need their k/q projections emitted first, PV groups need
            # the V chunks they read built first.
            EXTRAS_PER_SLOT = 1
            TAIL_OFFS = (0, 1, 2, 4, 6, 8, 10)
            slot = 0
            pvq = []     # pending (st, g, emit_slot) PV groups, in order
            tails = []   # (st, step, due_slot)
            done = {"k": 0, "q": 0, "vb": 0}  # kproj tb<=, qproj tb<=, vbuilds

            def pop_extra():
                fn, *args = extras.pop(0)
                fn(*args)
                if fn is proj:
                    if args[0] == 1:
                        done["k"] = max(done["k"], max(args[1]))
                    elif args[0] == 0:
                        done["q"] = max(done["q"], max(args[1]))
                else:
                    done["vb"] += 4

            def pump_pv():
                for _ in range(2):
                    if not pvq:
                        return
                    s2, g2, es = pvq[0]
                    if slot < es + 2 or done["vb"] < min(N_CH, (3 * g2 + 2) // 2 + 1):
                        return
                    pvq.pop(0)
                    emit_pv(s2, g2)
                    if g2 == n_grp - 1:
                        for k, off in enumerate(TAIL_OFFS):
                            tails.append((s2, k, slot + off))
                        return  # drains must run before the next block's PV

            def pump_tails():
                while tails and tails[0][2] <= slot:
                    s2, k, _ = tails.pop(0)
                    tail_step(s2, k)

            for qb in range(N_QB):
                st = {"qb": qb, "pts": {}, "pv": None}
                for g in range(n_grp):
                    k_need = min(N_QB - 1, (3 * g + 2) // 8)
                    while extras and (done["k"] < k_need or done["q"] < qb):
                        pop_extra()
                    emit_scores_exp(st, g)
                    pvq.append((st, g, slot))
                    for _ in range(EXTRAS_PER_SLOT):
                        if extras:
                            pop_extra()
                    pump_pv()
                    pump_tails()
                    slot += 1
            while extras:
                pop_extra()
            while pvq or tails:
                pump_pv()
                pump_tails()
                slot += 1

    nc.compile()
    return nc


_NC = None
LAST = {}


def _get_nc():
    global _NC
    if _NC is None:
        _NC = _build()
    return _NC


def kernel(x, w_qkv, b_qkv, w_out, b_out):
    x = np.asarray(x, dtype=np.float32)
    w_qkv = np.asarray(w_qkv, dtype=np.float32)
    b_qkv = np.asarray(b_qkv, dtype=np.float32)
    w_out = np.asarray(w_out, dtype=np.float32)
    b_out = np.asarray(b_out, dtype=np.float32)

    bf = ml_dtypes.bfloat16
    in_maps = []
    for c in range(N_CORES):
        b = c // 4
        h0 = (c % 4) * HPC * D  # first head's column offset (2 heads = 128 cols)
        w_slice = np.concatenate(
            [w_qkv[:, j * E + h0 : j * E + h0 + HPC * D] for j in range(3)], axis=1
        )
        b_slice = np.concatenate(
            [b_qkv[j * E + h0 : j * E + h0 + HPC * D] for j in range(3)]
        )[:, None]
        in_maps.append(
            {
                "xt": np.ascontiguousarray(x[b].T).astype(bf),
                "wqkv": np.ascontiguousarray(w_slice).astype(bf),
                "bqkv": np.ascontiguousarray(b_slice.astype(np.float32)),
                "wout": np.ascontiguousarray(w_out[h0 : h0 + HPC * D, :]).astype(bf),
            }
        )

    res = run_bass_kernel_spmd(_get_nc(), in_maps, list(range(N_CORES)))
    LAST["exec_time_ns"] = res.exec_time_ns
    LAST["res"] = res

    out = np.empty((B, S, E), dtype=np.float32)
    for b in range(B):
        acc = res.results[4 * b]["out"].astype(np.float32)
        for c in range(4 * b + 1, 4 * b + 4):
            acc = acc + res.results[c]["out"]
        out[b] = acc.T + b_out[None, :]
    return out
